# revision 32
# baseline (speedup 1.0000x reference)
"""Trainium2 Bass kernel for nn_AttentionVTP (8-core batch-parallel).

Per core = one batch element. Two outputs: y [b,n,512] and softmax(dots0) [b,h,n,n].
Host precomputes expB = exp(rpb + 0.01*pos) (batch-independent); device does
scores->top-k masks->weight gathers, qkv, attention with fused softmax, second
pruning, output projection.
"""
import numpy as np

B, N, D, H, DH = 8, 1024, 512, 8, 64
KD = 409           # kept input channels after pruning 1
K2 = 51            # kept per-head channels after pruning 2
SCALE = DH ** -0.5
NCH = N // 128     # 8 n-chunks
DCH = D // 128     # 4 d-chunks
GI = 2             # i-blocks (of 128 query rows) per attention group
NGRP = NCH // GI

MM_MODE = "f32r"   # "f32" (exact, 4cyc/row) or "f32r" (tf32-ish, 1cyc/row @N>=256)

_CACHE = {}


def _build_nc():
    import concourse.mybir as mybir
    import concourse.tile as tile
    from concourse import bacc
    from concourse.bass import IndirectOffsetOnAxis

    f32 = mybir.dt.float32
    i32 = mybir.dt.int32
    MMDT = mybir.dt.float32r if MM_MODE == "f32r" else f32
    Alu = mybir.AluOpType
    Act = mybir.ActivationFunctionType
    AX = mybir.AxisListType.X

    nc = bacc.Bacc(None, target_bir_lowering=False, debug=False)

    with tile.TileContext(nc) as tc:
        with tc.tile_pool(name="dram", bufs=1, space="DRAM") as dram:
            def din(name, shape, dt=f32):
                return dram.tile(shape, dt, kind="ExternalInput", name=name, uniquify=False)

            x_d = din("x", [N, D])
            wsx_d = din("wsx", [N, 1])
            wqkv_d = din("wqkv_pad", [KD + 1, 3 * D])
            wout_d = din("wout_pad", [H * K2 + 1, D])
            expb_d = din("expb", [H, N, N])
            wsb_d = din("wscores_b", [128, N])
            boutb_d = din("bout_b", [128, D])
            lt1_d = din("lt1", [D, D])
            lt2_d = din("lt2", [D, D])
            hb_d = din("hb", [128, DCH])
            idt_d = din("idt", [128, 128])
            y_d = dram.tile([N, D], f32, kind="ExternalOutput", name="y", uniquify=False)
            a0_d = dram.tile([H, N, N], f32, kind="ExternalOutput", name="attn0", uniquify=False)
            sums_d = dram.tile([H * N], f32, kind="Internal", name="sums_scratch")

            with tc.tile_pool(name="persist", bufs=1) as pp:
                # ---- persistent SBUF ----
                wsx_sb = pp.tile([128, NCH], f32, name="wsx_sb")
                nc.sync.dma_start(out=wsx_sb[:], in_=wsx_d[:].rearrange("(i p) o -> p (i o)", p=128))
                wsb_sb = pp.tile([128, N], f32, name="wsb_sb")
                nc.sync.dma_start(out=wsb_sb[:], in_=wsb_d[:])
                boutb_sb = pp.tile([128, D], f32, name="boutb_sb")
                nc.sync.dma_start(out=boutb_sb[:], in_=boutb_d[:])
                hb_sb = pp.tile([128, DCH], f32, name="hb_sb")
                nc.sync.dma_start(out=hb_sb[:], in_=hb_d[:])
                ident = pp.tile([128, 128], f32, name="ident")
                nc.sync.dma_start(out=ident[:], in_=idt_d[:])
                lt2_sb = pp.tile([128, DCH, D], f32, name="lt2_sb")
                nc.sync.dma_start(out=lt2_sb[:], in_=lt2_d[:].rearrange("(j p) c -> p j c", p=128))

                ones64 = pp.tile([1, 64], f32, name="ones64")
                nc.vector.memset(ones64[:], 1.0)
                c409 = pp.tile([128, 1], f32, name="c409")
                nc.vector.memset(c409[:], float(KD))
                c408 = pp.tile([128, 1], f32, name="c408")
                nc.vector.memset(c408[:], float(H * K2))
                c1 = pp.tile([128, 1], f32, name="c1")
                nc.vector.memset(c1[:], 1.0)

                qkT_sb = [pp.tile([128, N], MMDT, name=f"qkT{m}") for m in range(8)]
                # v with a ones column per head: [nk-chunk][128, H, DH+1]
                v_sb = [pp.tile([128, H, DH + 1], MMDT, name=f"v{i}") for i in range(NCH)]
                outT_sb = [pp.tile([128, N], f32, name=f"outT{c}") for c in range(DCH)]
                sums0 = pp.tile([128, H * NCH], f32, name="sums0")

                # early pool for expb prefetch (lowest addresses -> no false dep on phase A)
                pool_expb = tc.tile_pool(name="pb_expb", bufs=2)
                pbx = pool_expb.__enter__()

                # ================= Phase A =================
                with tc.tile_pool(name="pa_sb", bufs=1) as pa:
                    x_sb = pa.tile([128, NCH, D], f32, name="x_sb")
                    nc.sync.dma_start(out=x_sb[:], in_=x_d[:].rearrange("(i p) d -> p i d", p=128))
                    lt1_sb = pa.tile([128, DCH, D], f32, name="lt1_sb")
                    nc.sync.dma_start(out=lt1_sb[:], in_=lt1_d[:].rearrange("(j p) c -> p j c", p=128))

                    with tc.tile_pool(name="pa_ps1", bufs=1, space="PSUM") as pap:
                        # s1[c] = sum_n x[n, c] * wsx[n]
                        s1_ps = pap.tile([128, DCH], f32, space="PSUM", name="s1_ps")
                        for j in range(DCH):
                            for i in range(NCH):
                                nc.tensor.matmul(out=s1_ps[:, j:j + 1],
                                                 lhsT=x_sb[:, i, j * 128:(j + 1) * 128],
                                                 rhs=wsx_sb[:, i:i + 1],
                                                 start=(i == 0), stop=(i == NCH - 1))
                        s1_sb = pa.tile([128, DCH], f32, name="s1_sb")
                        nc.vector.tensor_copy(out=s1_sb[:], in_=s1_ps[:])

                        # s1 as a broadcast row [128, 512]
                        s1row_sb = pa.tile([1, D], f32, name="s1row_sb")
                        for j in range(DCH):
                            row_ps = pap.tile([1, 128], f32, space="PSUM", name="row_ps", tag="row", bufs=2)
                            nc.tensor.transpose(out=row_ps[:], in_=s1_sb[:, j:j + 1], identity=ident[:])
                            nc.vector.tensor_copy(out=s1row_sb[:, j * 128:(j + 1) * 128], in_=row_ps[:])
                        ones128 = pa.tile([1, 128], f32, name="ones128")
                        nc.vector.memset(ones128[:], 1.0)
                        s1rb_ps = pap.tile([128, D], f32, space="PSUM", name="s1rb_ps")
                        nc.tensor.matmul(out=s1rb_ps[:], lhsT=ones128[:], rhs=s1row_sb[:], start=True, stop=True)
                        s1rb_sb = pa.tile([128, D], f32, name="s1rb_sb")
                        nc.vector.tensor_copy(out=s1rb_sb[:], in_=s1rb_ps[:])

                        # greater-count -> mask1
                        scratch = pa.tile([128, D], f32, name="scratch")
                        cnt_sb = pa.tile([128, DCH], f32, name="cnt_sb")
                        mask1 = pa.tile([128, DCH], f32, name="mask1")
                        for j in range(DCH):
                            nc.vector.tensor_tensor(
                                out=scratch[:], in0=s1rb_sb[:],
                                in1=s1_sb[:, j:j + 1].to_broadcast([128, D]), op=Alu.is_gt)
                            nc.vector.reduce_sum(out=cnt_sb[:, j:j + 1], in_=scratch[:], axis=AX)
                            nc.vector.tensor_scalar(out=mask1[:, j:j + 1], in0=cnt_sb[:, j:j + 1],
                                                    scalar1=float(KD), scalar2=None, op0=Alu.is_lt)

                        # rank1 = strict-lower-tri @ mask1 ; g1 = select(mask1, rank1, 409)
                        r1_ps = pap.tile([128, DCH], f32, space="PSUM", name="r1_ps")
                        mask1i = pa.tile([128, DCH], i32, name="mask1i")
                        nc.vector.tensor_copy(out=mask1i[:], in_=mask1[:])
                        g1f = pa.tile([128, DCH], f32, name="g1f")
                        g1i = pa.tile([128, DCH], i32, name="g1i")
                        for i in range(DCH):
                            for j in range(i + 1):
                                nc.tensor.matmul(out=r1_ps[:, i:i + 1],
                                                 lhsT=lt1_sb[:, j, i * 128:(i + 1) * 128],
                                                 rhs=mask1[:, j:j + 1],
                                                 start=(j == 0), stop=(j == i))
                            nc.vector.select(out=g1f[:, i:i + 1], mask=mask1i[:, i:i + 1],
                                             on_true=r1_ps[:, i:i + 1], on_false=c409[:])
                        nc.vector.tensor_copy(out=g1i[:], in_=g1f[:])

                    # gather W_sel rows (into f32 temp, round-copy to MMDT)
                    wsel_r = pa.tile([128, DCH, 3 * D], MMDT, name="wsel_r")
                    with tc.tile_pool(name="pa_gt", bufs=2) as pagt:
                        for j in range(DCH):
                            gtmp = pagt.tile([128, 3 * D], f32, name="gtmp", tag="gtmp")
                            nc.gpsimd.indirect_dma_start(
                                out=gtmp[:], out_offset=None,
                                in_=wqkv_d[:],
                                in_offset=IndirectOffsetOnAxis(ap=g1i[:, j:j + 1], axis=0))
                            nc.vector.tensor_copy(out=wsel_r[:, j, :], in_=gtmp[:])

                    # xT tiles [d, n]
                    xT_sb = pa.tile([128, DCH, N], MMDT, name="xT_sb")
                    with tc.tile_pool(name="pa_tp", bufs=2, space="PSUM") as patp:
                        for i in range(NCH):
                            for j in range(DCH):
                                tp = patp.tile([128, 128], f32, space="PSUM", name="tp")
                                nc.tensor.transpose(out=tp[:], in_=x_sb[:, i, j * 128:(j + 1) * 128],
                                                    identity=ident[:])
                                if (i * DCH + j) % 2 == 0:
                                    nc.vector.tensor_copy(out=xT_sb[:, j, i * 128:(i + 1) * 128], in_=tp[:])
                                else:
                                    nc.scalar.copy(out=xT_sb[:, j, i * 128:(i + 1) * 128], in_=tp[:])

                    # qT/kT: [e, n] = sum_d wsel[d, e] * xT[d, n]
                    with tc.tile_pool(name="pa_mm", bufs=2, space="PSUM") as pam:
                        for m in range(8):
                            for half in range(2):
                                mm_ps = pam.tile([128, 512], f32, space="PSUM", name="mm_ps")
                                for j in range(DCH):
                                    nc.tensor.matmul(out=mm_ps[:],
                                                     lhsT=wsel_r[:, j, m * 128:(m + 1) * 128],
                                                     rhs=xT_sb[:, j, half * 512:(half + 1) * 512],
                                                     start=(j == 0), stop=(j == DCH - 1))
                                if m % 2 == 0:
                                    nc.vector.tensor_copy(out=qkT_sb[m][:, half * 512:(half + 1) * 512], in_=mm_ps[:])
                                else:
                                    nc.scalar.copy(out=qkT_sb[m][:, half * 512:(half + 1) * 512], in_=mm_ps[:])
                        # v natural: [n, dh_all]; write into [128, H, DH+1] layout + ones col
                        for i in range(NCH):
                            mm_ps = pam.tile([128, 512], f32, space="PSUM", name="mm_ps")
                            for j in range(DCH):
                                nc.tensor.matmul(out=mm_ps[:],
                                                 lhsT=xT_sb[:, j, i * 128:(i + 1) * 128],
                                                 rhs=wsel_r[:, j, 2 * D:3 * D],
                                                 start=(j == 0), stop=(j == DCH - 1))
                            if i % 2 == 0:
                                nc.vector.tensor_copy(out=v_sb[i][:, :, 0:DH],
                                                      in_=mm_ps[:].rearrange("p (h e) -> p h e", h=H))
                            else:
                                nc.scalar.copy(out=v_sb[i][:, :, 0:DH],
                                               in_=mm_ps[:].rearrange("p (h e) -> p h e", h=H))
                            nc.vector.tensor_copy(out=v_sb[i][:, :, DH:DH + 1],
                                                  in_=c1[:].to_broadcast([128, H, 1]))

                # ================= Phase B: attention =================
                def qT(h):
                    t = qkT_sb[h // 2]
                    p0 = (h % 2) * 64
                    return t[p0:p0 + 64, :]

                def kT(h):
                    t = qkT_sb[4 + h // 2]
                    p0 = (h % 2) * 64
                    return t[p0:p0 + 64, :]

                with tc.tile_pool(name="pb_sb", bufs=2) as pb, \
                     tc.tile_pool(name="pb_e1t", bufs=2) as pbe, \
                     tc.tile_pool(name="pb_dots", bufs=2, space="PSUM") as pbd, \
                     tc.tile_pool(name="pb_tp", bufs=2, space="PSUM") as pbt, \
                     tc.tile_pool(name="pb_av", bufs=2, space="PSUM") as pba:
                    cpi = 0
                    for h in range(H):
                        for g in range(NGRP):
                            expb_t = pbx.tile([128, GI, N], f32, name="expb_t", tag="expb")
                            nc.sync.dma_start(
                                out=expb_t[:],
                                in_=expb_d[h, g * GI * 128:(g + 1) * GI * 128, :]
                                    .rearrange("(ii p) m -> p ii m", p=128))
                            e0_t = pb.tile([128, GI, N], f32, name="e0_t", tag="e0")
                            e1_t = pb.tile([128, GI, N], f32, name="e1_t", tag="e1")
                            a0_t = pb.tile([128, GI, N], f32, name="a0_t", tag="a0")
                            rcp0_t = pb.tile([128, GI], f32, name="rcp0_t", tag="rcp0")
                            e1T_t = pbe.tile([128, NCH, GI * 128], MMDT, name="e1T_t", tag="e1T")

                            for ii in range(GI):
                                i = g * GI + ii
                                col = h * NCH + i
                                dots_ps = pbd.tile([128, N], f32, space="PSUM", name="dots_ps", tag="dots")
                                for half in range(2):
                                    nc.tensor.matmul(out=dots_ps[:, half * 512:(half + 1) * 512],
                                                     lhsT=qT(h)[:, i * 128:(i + 1) * 128],
                                                     rhs=kT(h)[:, half * 512:(half + 1) * 512],
                                                     start=True, stop=True)
                                nc.scalar.activation(out=e0_t[:, ii, :], in_=dots_ps[:],
                                                     func=Act.Exp, scale=SCALE,
                                                     accum_out=sums0[:, col:col + 1])
                                nc.vector.tensor_tensor(out=e1_t[:, ii, :], in0=e0_t[:, ii, :],
                                                        in1=expb_t[:, ii, :], op=Alu.mult)
                                nc.vector.reciprocal(out=rcp0_t[:, ii:ii + 1], in_=sums0[:, col:col + 1])
                                if (h * NGRP + g) % 2 == 0:
                                    nc.vector.tensor_scalar(out=a0_t[:, ii, :], in0=e0_t[:, ii, :],
                                                            scalar1=rcp0_t[:, ii:ii + 1], scalar2=None,
                                                            op0=Alu.mult)
                                else:
                                    nc.scalar.activation(out=a0_t[:, ii, :], in_=e0_t[:, ii, :],
                                                         func=Act.Copy, bias=0.0,
                                                         scale=rcp0_t[:, ii:ii + 1])
                                for kcg in range(2):
                                    tp = pbt.tile([128, 512], f32, space="PSUM", name="tpb", tag="tpb")
                                    for q in range(4):
                                        kc = kcg * 4 + q
                                        nc.tensor.transpose(
                                            out=tp[:, q * 128:(q + 1) * 128],
                                            in_=e1_t[:, ii, kc * 128:(kc + 1) * 128],
                                            identity=ident[:])
                                    dst = e1T_t[:, kcg * 4:(kcg + 1) * 4, ii * 128:(ii + 1) * 128]
                                    src = tp[:].rearrange("p (q c) -> p q c", q=4)
                                    if cpi % 2 == 0:
                                        nc.vector.tensor_copy(out=dst, in_=src)
                                    else:
                                        nc.scalar.copy(out=dst, in_=src)
                                    cpi += 1
                            # attn0 out on the ACT HWDGE ring (loads use the SP ring)
                            nc.scalar.dma_start(
                                out=a0_d[h, g * GI * 128:(g + 1) * GI * 128, :]
                                    .rearrange("(ii p) m -> p ii m", p=128),
                                in_=a0_t[:])
                            # attn @ v -> out^T [dh+1, nq-group]; row DH = sums1
                            av_ps = pba.tile([DH + 1, GI * 128], f32, space="PSUM", name="av_ps", tag="av")
                            for kc in range(NCH):
                                nc.tensor.matmul(out=av_ps[:],
                                                 lhsT=v_sb[kc][:, h, :],
                                                 rhs=e1T_t[:, kc, :],
                                                 start=(kc == 0), stop=(kc == NCH - 1))
                            p0 = (h % 2) * 64
                            nc.scalar.copy(out=outT_sb[h // 2][p0:p0 + 64, g * GI * 128:(g + 1) * GI * 128],
                                           in_=av_ps[0:DH, :])
                            sstage = pb.tile([1, GI * 128], f32, name="sstage", tag="sstage")
                            nc.scalar.copy(out=sstage[:], in_=av_ps[DH:DH + 1, :])
                            nc.gpsimd.dma_start(
                                out=sums_d[h * N + g * GI * 128:h * N + (g + 1) * GI * 128],
                                in_=sstage[:])

                # ================= Phase C =================
                with tc.tile_pool(name="pc_sb", bufs=1) as pc:
                    # sums scratch [H*N] -> [128, 64] -> reciprocal -> [64, 128] -> flat row
                    sums_pk = pc.tile([128, H * NCH], f32, name="sums_pk")
                    nc.sync.dma_start(
                        out=sums_pk[:],
                        in_=sums_d[:].rearrange("(r c) -> c r", r=H * NCH))
                    rcp_all = pc.tile([128, H * NCH], f32, name="rcp_all")
                    nc.vector.reciprocal(out=rcp_all[:], in_=sums_pk[:])
                    rcpT_sb = pc.tile([64, 128], f32, name="rcpT_sb")
                    with tc.tile_pool(name="pc_ps0", bufs=1, space="PSUM") as pcp0:
                        rcpT_ps = pcp0.tile([64, 128], f32, space="PSUM", name="rcpT_ps")
                        nc.tensor.transpose(out=rcpT_ps[:], in_=rcp_all[:], identity=ident[:])
                        nc.vector.tensor_copy(out=rcpT_sb[:], in_=rcpT_ps[:])
                    rcpflat = pc.tile([1, 64 * 128], f32, name="rcpflat")
                    nc.sync.dma_start(
                        out=rcpflat[:].rearrange("p (r c) -> p r c", r=64),
                        in_=rcpT_sb[:])

                    # rcp1 broadcast; scale outT; s2
                    outTs = [pc.tile([128, N], MMDT, name=f"outTs{c}") for c in range(DCH)]
                    s2cols = pc.tile([128, DCH], f32, name="s2cols")
                    scr2 = pc.tile([128, N], f32, name="scr2")
                    with tc.tile_pool(name="pc_rb", bufs=2, space="PSUM") as pcrb:
                        for ch in range(DCH):
                            rb_ps = pcrb.tile([128, N], f32, space="PSUM", name="rb_ps", tag="rb")
                            for hh in range(2):
                                hcur = ch * 2 + hh
                                for i in range(NCH):
                                    r = hcur * NCH + i
                                    nc.tensor.matmul(
                                        out=rb_ps[hh * 64:hh * 64 + 64, i * 128:(i + 1) * 128],
                                        lhsT=ones64[:],
                                        rhs=rcpflat[:, r * 128:(r + 1) * 128],
                                        start=True, stop=True)
                            rb_sb = pc.tile([128, N], f32, name="rb_sb", tag="rbsb", bufs=2)
                            nc.vector.tensor_copy(out=rb_sb[:], in_=rb_ps[:])
                            outTn = pc.tile([128, N], f32, name="outTn", tag="outTn", bufs=2)
                            nc.vector.tensor_tensor(out=outTn[:], in0=outT_sb[ch][:],
                                                    in1=rb_sb[:], op=Alu.mult)
                            nc.vector.tensor_copy(out=outTs[ch][:], in_=outTn[:])
                            nc.vector.tensor_tensor(out=scr2[:], in0=outTn[:], in1=wsb_sb[:], op=Alu.mult)
                            nc.vector.reduce_sum(out=s2cols[:, ch:ch + 1], in_=scr2[:], axis=AX)

                    # s2 row + per-head compare table
                    s2row_sb = pc.tile([1, D], f32, name="s2row_sb")
                    with tc.tile_pool(name="pc_ps1", bufs=2, space="PSUM") as pcp1:
                        for j in range(DCH):
                            s2row_ps = pcp1.tile([1, 128], f32, space="PSUM", name="s2row_ps", tag="s2r")
                            nc.tensor.transpose(out=s2row_ps[:], in_=s2cols[:, j:j + 1], identity=ident[:])
                            nc.vector.tensor_copy(out=s2row_sb[:, j * 128:(j + 1) * 128], in_=s2row_ps[:])

                    cnt2 = pc.tile([128, DCH], f32, name="cnt2")
                    mask2 = pc.tile([128, DCH], f32, name="mask2")
                    g2f = pc.tile([128, DCH], f32, name="g2f")
                    g2i = pc.tile([128, DCH], i32, name="g2i")
                    scr64 = pc.tile([128, 64], f32, name="scr64")
                    r2add = pc.tile([128, DCH], f32, name="r2add")
                    with tc.tile_pool(name="pc_t2", bufs=2, space="PSUM") as pct2:
                        for ch in range(DCH):
                            t2_ps = pct2.tile([128, 64], f32, space="PSUM", name="t2_ps", tag="t2")
                            for hh in range(2):
                                hcur = ch * 2 + hh
                                nc.tensor.matmul(out=t2_ps[hh * 64:hh * 64 + 64, :],
                                                 lhsT=ones64[:],
                                                 rhs=s2row_sb[:, hcur * 64:(hcur + 1) * 64],
                                                 start=True, stop=True)
                            nc.vector.tensor_tensor(
                                out=scr64[:], in0=t2_ps[:],
                                in1=s2cols[:, ch:ch + 1].to_broadcast([128, 64]), op=Alu.is_gt)
                            nc.vector.reduce_sum(out=cnt2[:, ch:ch + 1], in_=scr64[:], axis=AX)
                            nc.vector.tensor_scalar(out=mask2[:, ch:ch + 1], in0=cnt2[:, ch:ch + 1],
                                                    scalar1=float(K2), scalar2=None, op0=Alu.is_lt)

                    with tc.tile_pool(name="pc_ps2", bufs=1, space="PSUM") as pcp2:
                        r2_ps = pcp2.tile([128, DCH], f32, space="PSUM", name="r2_ps")
                        mask2i = pc.tile([128, DCH], i32, name="mask2i")
                        nc.vector.tensor_copy(out=mask2i[:], in_=mask2[:])
                        for ch in range(DCH):
                            nc.tensor.matmul(out=r2_ps[:, ch:ch + 1],
                                             lhsT=lt2_sb[:, ch, ch * 128:(ch + 1) * 128],
                                             rhs=mask2[:, ch:ch + 1],
                                             start=True, stop=True)
                            nc.vector.tensor_tensor(out=r2add[:, ch:ch + 1], in0=r2_ps[:, ch:ch + 1],
                                                    in1=hb_sb[:, ch:ch + 1], op=Alu.add)
                            nc.vector.select(out=g2f[:, ch:ch + 1], mask=mask2i[:, ch:ch + 1],
                                             on_true=r2add[:, ch:ch + 1], on_false=c408[:])
                        nc.vector.tensor_copy(out=g2i[:], in_=g2f[:])

                    wsel2_r = pc.tile([128, DCH, D], MMDT, name="wsel2_r")
                    with tc.tile_pool(name="pc_gt", bufs=2) as pcgt:
                        for j in range(DCH):
                            gtmp2 = pcgt.tile([128, D], f32, name="gtmp2", tag="gtmp2")
                            nc.gpsimd.indirect_dma_start(
                                out=gtmp2[:], out_offset=None,
                                in_=wout_d[:],
                                in_offset=IndirectOffsetOnAxis(ap=g2i[:, j:j + 1], axis=0))
                            nc.vector.tensor_copy(out=wsel2_r[:, j, :], in_=gtmp2[:])

                    # y = out2norm @ W_out_sel + b_out
                    with tc.tile_pool(name="pc_y", bufs=2, space="PSUM") as pcy:
                        for i in range(NCH):
                            y_ps = pcy.tile([128, D], f32, space="PSUM", name="y_ps", tag="y")
                            for j in range(DCH):
                                nc.tensor.matmul(out=y_ps[:],
                                                 lhsT=outTs[j][:, i * 128:(i + 1) * 128],
                                                 rhs=wsel2_r[:, j, :],
                                                 start=(j == 0), stop=(j == DCH - 1))
                            y_sb = pc.tile([128, D], f32, name="y_sb", tag="ysb", bufs=2)
                            nc.vector.tensor_tensor(out=y_sb[:], in0=y_ps[:], in1=boutb_sb[:], op=Alu.add)
                            nc.scalar.dma_start(out=y_d[i * 128:(i + 1) * 128, :], in_=y_sb[:])

                pool_expb.__exit__(None, None, None)

    nc.compile()
    return nc


def _host_prep(W_scoresx, W_qkv, rpb_table, headsita, W_scores, W_out, b_out, rel_index, dis):
    """Precompute batch-independent tensors on the host."""
    f = np.float32
    rpb = rpb_table[rel_index.reshape(-1)].reshape(N, N, H).transpose(2, 0, 1).astype(np.float64)
    factor = 1.0 / (2.0 * headsita.astype(np.float64) ** 2 + 1e-10)
    pos = np.exp(-factor[:, None, None] * dis.astype(np.float64)[None, :, :])
    expb = np.exp(rpb + 0.01 * pos).astype(f)

    wqkv_pad = np.zeros((KD + 1, 3 * D), f)
    wqkv_pad[:KD] = W_qkv
    wout_pad = np.zeros((H * K2 + 1, D), f)
    wout_pad[:H * K2] = W_out
    wsb = np.broadcast_to(W_scores.reshape(1, N), (128, N)).astype(f).copy()
    boutb = np.broadcast_to(b_out.reshape(1, D), (128, D)).astype(f).copy()
    lt1 = np.triu(np.ones((D, D), f), k=1)          # lt1[c', c] = 1 if c' < c
    lt2 = np.zeros((D, D), f)
    for h in range(H):
        lt2[h * DH:(h + 1) * DH, h * DH:(h + 1) * DH] = np.triu(np.ones((DH, DH), f), k=1)
    hb = np.zeros((128, DCH), f)
    for p in range(128):
        for j in range(DCH):
            hb[p, j] = ((j * 128 + p) // DH) * K2
    return {
        "wsx": W_scoresx.astype(f).reshape(N, 1),
        "wqkv_pad": wqkv_pad, "wout_pad": wout_pad, "expb": expb,
        "wscores_b": wsb, "bout_b": boutb, "lt1": lt1, "lt2": lt2, "hb": hb,
        "idt": np.eye(128, dtype=f),
    }


def kernel(x, W_scoresx, b_scoresx, W_qkv, rpb_table, headsita, W_scores, b_scores,
           W_out, b_out, rel_index, dis, _trace=False):
    from concourse.bass_utils import run_bass_kernel_spmd

    x = np.ascontiguousarray(np.asarray(x, dtype=np.float32))
    shared = _host_prep(np.asarray(W_scoresx, np.float32), np.asarray(W_qkv, np.float32),
                        np.asarray(rpb_table, np.float32), np.asarray(headsita, np.float32),
                        np.asarray(W_scores, np.float32), np.asarray(W_out, np.float32),
                        np.asarray(b_out, np.float32), np.asarray(rel_index),
                        np.asarray(dis, np.float32))

    if "nc" not in _CACHE:
        _CACHE["nc"] = _build_nc()
    nc = _CACHE["nc"]

    in_maps = []
    for c in range(B):
        m = dict(shared)
        m["x"] = np.ascontiguousarray(x[c])
        in_maps.append(m)
    res = run_bass_kernel_spmd(nc, in_maps, core_ids=list(range(B)), trace=_trace)
    y = np.stack([res.results[c]["y"] for c in range(B)])
    attn0 = np.stack([res.results[c]["attn0"] for c in range(B)])
    if _trace:
        _CACHE["last_result"] = res
    return y, attn0


# revision 33
# speedup vs baseline: 1.1389x; 1.1389x over previous
"""Trainium2 Bass kernel for nn_AttentionVTP (8-core batch-parallel).

Per core = one batch element. Two outputs: y [b,n,512] and softmax(dots0) [b,h,n,n].
Host precomputes expB = exp(rpb + 0.01*pos) (batch-independent); device does
scores->top-k masks->weight gathers, qkv, attention with fused softmax, second
pruning, output projection.
"""
import numpy as np

B, N, D, H, DH = 8, 1024, 512, 8, 64
KD = 409           # kept input channels after pruning 1
K2 = 51            # kept per-head channels after pruning 2
SCALE = DH ** -0.5
NCH = N // 128     # 8 n-chunks
DCH = D // 128     # 4 d-chunks
GI = 2             # i-blocks (of 128 query rows) per attention group
NGRP = NCH // GI

MM_MODE = "f32r"   # "f32" (exact, 4cyc/row) or "f32r" (tf32-ish, 1cyc/row @N>=256)

_CACHE = {}


def _build_nc():
    import concourse.mybir as mybir
    import concourse.tile as tile
    from concourse import bacc
    from concourse.bass import IndirectOffsetOnAxis

    f32 = mybir.dt.float32
    i32 = mybir.dt.int32
    MMDT = mybir.dt.float32r if MM_MODE == "f32r" else f32
    Alu = mybir.AluOpType
    Act = mybir.ActivationFunctionType
    AX = mybir.AxisListType.X

    nc = bacc.Bacc(None, target_bir_lowering=False, debug=False)

    with tile.TileContext(nc) as tc:
        with tc.tile_pool(name="dram", bufs=1, space="DRAM") as dram:
            def din(name, shape, dt=f32):
                return dram.tile(shape, dt, kind="ExternalInput", name=name, uniquify=False)

            x_d = din("x", [N, D])
            wsx_d = din("wsx", [N, 1])
            wqkv_d = din("wqkv_pad", [KD + 1, 3 * D])
            wout_d = din("wout_pad", [H * K2 + 1, D])
            expb_d = din("expb", [H, N, N])
            wsb_d = din("wscores_b", [128, N])
            boutb_d = din("bout_b", [128, D])
            lt1_d = din("lt1", [D, D])
            lt2_d = din("lt2", [D, D])
            hb_d = din("hb", [128, DCH])
            idt_d = din("idt", [128, 128])
            y_d = dram.tile([N, D], f32, kind="ExternalOutput", name="y", uniquify=False)
            a0_d = dram.tile([H, N, N], f32, kind="ExternalOutput", name="attn0", uniquify=False)
            sums_d = dram.tile([H * N], f32, kind="Internal", name="sums_scratch")

            with tc.tile_pool(name="persist", bufs=1) as pp:
                # ---- persistent SBUF ----
                wsx_sb = pp.tile([128, NCH], f32, name="wsx_sb")
                nc.sync.dma_start(out=wsx_sb[:], in_=wsx_d[:].rearrange("(i p) o -> p (i o)", p=128))
                wsb_sb = pp.tile([128, N], f32, name="wsb_sb")
                nc.sync.dma_start(out=wsb_sb[:], in_=wsb_d[:])
                boutb_sb = pp.tile([128, D], f32, name="boutb_sb")
                nc.sync.dma_start(out=boutb_sb[:], in_=boutb_d[:])
                hb_sb = pp.tile([128, DCH], f32, name="hb_sb")
                nc.sync.dma_start(out=hb_sb[:], in_=hb_d[:])
                ident = pp.tile([128, 128], f32, name="ident")
                nc.sync.dma_start(out=ident[:], in_=idt_d[:])
                lt2_sb = pp.tile([128, DCH, D], f32, name="lt2_sb")
                nc.sync.dma_start(out=lt2_sb[:], in_=lt2_d[:].rearrange("(j p) c -> p j c", p=128))

                ones64 = pp.tile([1, 64], f32, name="ones64")
                nc.vector.memset(ones64[:], 1.0)
                c409 = pp.tile([128, 1], f32, name="c409")
                nc.vector.memset(c409[:], float(KD))
                c408 = pp.tile([128, 1], f32, name="c408")
                nc.vector.memset(c408[:], float(H * K2))
                c1 = pp.tile([128, 1], f32, name="c1")
                nc.vector.memset(c1[:], 1.0)

                qkT_sb = [pp.tile([128, N], MMDT, name=f"qkT{m}") for m in range(8)]
                # v with a ones column per head: [nk-chunk][128, H, DH+1]
                v_sb = [pp.tile([128, H, DH + 1], MMDT, name=f"v{i}") for i in range(NCH)]
                outT_sb = [pp.tile([128, N], f32, name=f"outT{c}") for c in range(DCH)]
                sums0 = pp.tile([128, H * NCH], f32, name="sums0")

                # early pool for expb prefetch (lowest addresses -> no false dep on phase A)
                pool_expb = tc.tile_pool(name="pb_expb", bufs=2)
                pbx = pool_expb.__enter__()

                # ================= Phase A =================
                with tc.tile_pool(name="pa_sb", bufs=1) as pa:
                    x_sb = pa.tile([128, NCH, D], f32, name="x_sb")
                    nc.sync.dma_start(out=x_sb[:], in_=x_d[:].rearrange("(i p) d -> p i d", p=128))
                    lt1_sb = pa.tile([128, DCH, D], f32, name="lt1_sb")
                    nc.sync.dma_start(out=lt1_sb[:], in_=lt1_d[:].rearrange("(j p) c -> p j c", p=128))

                    with tc.tile_pool(name="pa_ps1", bufs=1, space="PSUM") as pap:
                        # s1[c] = sum_n x[n, c] * wsx[n]
                        s1_ps = pap.tile([128, DCH], f32, space="PSUM", name="s1_ps")
                        for j in range(DCH):
                            for i in range(NCH):
                                nc.tensor.matmul(out=s1_ps[:, j:j + 1],
                                                 lhsT=x_sb[:, i, j * 128:(j + 1) * 128],
                                                 rhs=wsx_sb[:, i:i + 1],
                                                 start=(i == 0), stop=(i == NCH - 1))
                        s1_sb = pa.tile([128, DCH], f32, name="s1_sb")
                        nc.vector.tensor_copy(out=s1_sb[:], in_=s1_ps[:])

                        # s1 as a broadcast row [128, 512]
                        s1row_sb = pa.tile([1, D], f32, name="s1row_sb")
                        for j in range(DCH):
                            row_ps = pap.tile([1, 128], f32, space="PSUM", name="row_ps", tag="row", bufs=2)
                            nc.tensor.transpose(out=row_ps[:], in_=s1_sb[:, j:j + 1], identity=ident[:])
                            nc.vector.tensor_copy(out=s1row_sb[:, j * 128:(j + 1) * 128], in_=row_ps[:])
                        ones128 = pa.tile([1, 128], f32, name="ones128")
                        nc.vector.memset(ones128[:], 1.0)
                        s1rb_ps = pap.tile([128, D], f32, space="PSUM", name="s1rb_ps")
                        nc.tensor.matmul(out=s1rb_ps[:], lhsT=ones128[:], rhs=s1row_sb[:], start=True, stop=True)
                        s1rb_sb = pa.tile([128, D], f32, name="s1rb_sb")
                        nc.vector.tensor_copy(out=s1rb_sb[:], in_=s1rb_ps[:])

                        # greater-count -> mask1
                        scratch = pa.tile([128, D], f32, name="scratch")
                        cnt_sb = pa.tile([128, DCH], f32, name="cnt_sb")
                        mask1 = pa.tile([128, DCH], f32, name="mask1")
                        for j in range(DCH):
                            nc.vector.tensor_tensor(
                                out=scratch[:], in0=s1rb_sb[:],
                                in1=s1_sb[:, j:j + 1].to_broadcast([128, D]), op=Alu.is_gt)
                            nc.vector.reduce_sum(out=cnt_sb[:, j:j + 1], in_=scratch[:], axis=AX)
                            nc.vector.tensor_scalar(out=mask1[:, j:j + 1], in0=cnt_sb[:, j:j + 1],
                                                    scalar1=float(KD), scalar2=None, op0=Alu.is_lt)

                        # rank1 = strict-lower-tri @ mask1 ; g1 = select(mask1, rank1, 409)
                        r1_ps = pap.tile([128, DCH], f32, space="PSUM", name="r1_ps")
                        mask1i = pa.tile([128, DCH], i32, name="mask1i")
                        nc.vector.tensor_copy(out=mask1i[:], in_=mask1[:])
                        g1f = pa.tile([128, DCH], f32, name="g1f")
                        g1i = pa.tile([128, DCH], i32, name="g1i")
                        for i in range(DCH):
                            for j in range(i + 1):
                                nc.tensor.matmul(out=r1_ps[:, i:i + 1],
                                                 lhsT=lt1_sb[:, j, i * 128:(i + 1) * 128],
                                                 rhs=mask1[:, j:j + 1],
                                                 start=(j == 0), stop=(j == i))
                            nc.vector.select(out=g1f[:, i:i + 1], mask=mask1i[:, i:i + 1],
                                             on_true=r1_ps[:, i:i + 1], on_false=c409[:])
                        nc.vector.tensor_copy(out=g1i[:], in_=g1f[:])

                    # gather W_sel rows (into f32 temp, round-copy to MMDT)
                    wsel_r = pa.tile([128, DCH, 3 * D], MMDT, name="wsel_r")
                    with tc.tile_pool(name="pa_gt", bufs=2) as pagt:
                        for j in range(DCH):
                            gtmp = pagt.tile([128, 3 * D], f32, name="gtmp", tag="gtmp")
                            nc.gpsimd.indirect_dma_start(
                                out=gtmp[:], out_offset=None,
                                in_=wqkv_d[:],
                                in_offset=IndirectOffsetOnAxis(ap=g1i[:, j:j + 1], axis=0))
                            nc.vector.tensor_copy(out=wsel_r[:, j, :], in_=gtmp[:])

                    # xT tiles [d, n]
                    xT_sb = pa.tile([128, DCH, N], MMDT, name="xT_sb")
                    with tc.tile_pool(name="pa_tp", bufs=2, space="PSUM") as patp:
                        for i in range(NCH):
                            for j in range(DCH):
                                tp = patp.tile([128, 128], f32, space="PSUM", name="tp")
                                nc.tensor.transpose(out=tp[:], in_=x_sb[:, i, j * 128:(j + 1) * 128],
                                                    identity=ident[:])
                                if (i * DCH + j) % 2 == 0:
                                    nc.vector.tensor_copy(out=xT_sb[:, j, i * 128:(i + 1) * 128], in_=tp[:])
                                else:
                                    nc.scalar.copy(out=xT_sb[:, j, i * 128:(i + 1) * 128], in_=tp[:])

                    # qT/kT: [e, n] = sum_d wsel[d, e] * xT[d, n]
                    with tc.tile_pool(name="pa_mm", bufs=2, space="PSUM") as pam:
                        for m in range(8):
                            for half in range(2):
                                mm_ps = pam.tile([128, 512], f32, space="PSUM", name="mm_ps")
                                for j in range(DCH):
                                    nc.tensor.matmul(out=mm_ps[:],
                                                     lhsT=wsel_r[:, j, m * 128:(m + 1) * 128],
                                                     rhs=xT_sb[:, j, half * 512:(half + 1) * 512],
                                                     start=(j == 0), stop=(j == DCH - 1))
                                if m % 2 == 0:
                                    nc.vector.tensor_copy(out=qkT_sb[m][:, half * 512:(half + 1) * 512], in_=mm_ps[:])
                                else:
                                    nc.scalar.copy(out=qkT_sb[m][:, half * 512:(half + 1) * 512], in_=mm_ps[:])
                        # v natural: [n, dh_all]; write into [128, H, DH+1] layout + ones col
                        for i in range(NCH):
                            mm_ps = pam.tile([128, 512], f32, space="PSUM", name="mm_ps")
                            for j in range(DCH):
                                nc.tensor.matmul(out=mm_ps[:],
                                                 lhsT=xT_sb[:, j, i * 128:(i + 1) * 128],
                                                 rhs=wsel_r[:, j, 2 * D:3 * D],
                                                 start=(j == 0), stop=(j == DCH - 1))
                            if i % 2 == 0:
                                nc.vector.tensor_copy(out=v_sb[i][:, :, 0:DH],
                                                      in_=mm_ps[:].rearrange("p (h e) -> p h e", h=H))
                            else:
                                nc.scalar.copy(out=v_sb[i][:, :, 0:DH],
                                               in_=mm_ps[:].rearrange("p (h e) -> p h e", h=H))
                            nc.vector.tensor_copy(out=v_sb[i][:, :, DH:DH + 1],
                                                  in_=c1[:].to_broadcast([128, H, 1]))

                # ================= Phase B: attention =================
                def qT(h):
                    t = qkT_sb[h // 2]
                    p0 = (h % 2) * 64
                    return t[p0:p0 + 64, :]

                def kT(h):
                    t = qkT_sb[4 + h // 2]
                    p0 = (h % 2) * 64
                    return t[p0:p0 + 64, :]

                with tc.tile_pool(name="pb_sb", bufs=2) as pb, \
                     tc.tile_pool(name="pb_e1t", bufs=2) as pbe, \
                     tc.tile_pool(name="pb_dots", bufs=2, space="PSUM") as pbd, \
                     tc.tile_pool(name="pb_tp", bufs=2, space="PSUM") as pbt, \
                     tc.tile_pool(name="pb_av", bufs=2, space="PSUM") as pba:
                    cpi = 0
                    for h in range(H):
                        for g in range(NGRP):
                            expb_t = pbx.tile([128, GI, N], f32, name="expb_t", tag="expb")
                            nc.sync.dma_start(
                                out=expb_t[:],
                                in_=expb_d[h, g * GI * 128:(g + 1) * GI * 128, :]
                                    .rearrange("(ii p) m -> p ii m", p=128))
                            e0_t = pb.tile([128, GI, N], f32, name="e0_t", tag="e0")
                            e1_t = pb.tile([128, GI, N], f32, name="e1_t", tag="e1")
                            a0_t = pb.tile([128, GI, N], f32, name="a0_t", tag="a0")
                            rcp0_t = pb.tile([128, GI], f32, name="rcp0_t", tag="rcp0")
                            e1T_t = pbe.tile([128, NCH, GI * 128], MMDT, name="e1T_t", tag="e1T")

                            for ii in range(GI):
                                i = g * GI + ii
                                col = h * NCH + i
                                dots_ps = pbd.tile([128, N], f32, space="PSUM", name="dots_ps", tag="dots")
                                for half in range(2):
                                    nc.tensor.matmul(out=dots_ps[:, half * 512:(half + 1) * 512],
                                                     lhsT=qT(h)[:, i * 128:(i + 1) * 128],
                                                     rhs=kT(h)[:, half * 512:(half + 1) * 512],
                                                     start=True, stop=True)
                                nc.scalar.activation(out=e0_t[:, ii, :], in_=dots_ps[:],
                                                     func=Act.Exp, scale=SCALE,
                                                     accum_out=sums0[:, col:col + 1])
                                nc.vector.tensor_tensor(out=e1_t[:, ii, :], in0=e0_t[:, ii, :],
                                                        in1=expb_t[:, ii, :], op=Alu.mult)
                                nc.vector.reciprocal(out=rcp0_t[:, ii:ii + 1], in_=sums0[:, col:col + 1])
                                if (h * NGRP + g) % 2 == 0:
                                    nc.vector.tensor_scalar(out=a0_t[:, ii, :], in0=e0_t[:, ii, :],
                                                            scalar1=rcp0_t[:, ii:ii + 1], scalar2=None,
                                                            op0=Alu.mult)
                                else:
                                    nc.scalar.activation(out=a0_t[:, ii, :], in_=e0_t[:, ii, :],
                                                         func=Act.Copy, bias=0.0,
                                                         scale=rcp0_t[:, ii:ii + 1])
                                for kcg in range(2):
                                    tp = pbt.tile([128, 512], f32, space="PSUM", name="tpb", tag="tpb")
                                    for q in range(4):
                                        kc = kcg * 4 + q
                                        nc.tensor.transpose(
                                            out=tp[:, q * 128:(q + 1) * 128],
                                            in_=e1_t[:, ii, kc * 128:(kc + 1) * 128],
                                            identity=ident[:])
                                    dst = e1T_t[:, kcg * 4:(kcg + 1) * 4, ii * 128:(ii + 1) * 128]
                                    src = tp[:].rearrange("p (q c) -> p q c", q=4)
                                    if cpi % 2 == 0:
                                        nc.vector.tensor_copy(out=dst, in_=src)
                                    else:
                                        nc.scalar.copy(out=dst, in_=src)
                                    cpi += 1
                            # attn0 out via SWDGE
                            nc.gpsimd.dma_start(
                                out=a0_d[h, g * GI * 128:(g + 1) * GI * 128, :]
                                    .rearrange("(ii p) m -> p ii m", p=128),
                                in_=a0_t[:])
                            # attn @ v -> out^T [dh+1, nq-group]; row DH = sums1
                            av_ps = pba.tile([DH + 1, GI * 128], f32, space="PSUM", name="av_ps", tag="av")
                            for kc in range(NCH):
                                nc.tensor.matmul(out=av_ps[:],
                                                 lhsT=v_sb[kc][:, h, :],
                                                 rhs=e1T_t[:, kc, :],
                                                 start=(kc == 0), stop=(kc == NCH - 1))
                            p0 = (h % 2) * 64
                            nc.scalar.copy(out=outT_sb[h // 2][p0:p0 + 64, g * GI * 128:(g + 1) * GI * 128],
                                           in_=av_ps[0:DH, :])
                            sstage = pb.tile([1, GI * 128], f32, name="sstage", tag="sstage")
                            nc.scalar.copy(out=sstage[:], in_=av_ps[DH:DH + 1, :])
                            nc.gpsimd.dma_start(
                                out=sums_d[h * N + g * GI * 128:h * N + (g + 1) * GI * 128],
                                in_=sstage[:])

                # ================= Phase C =================
                with tc.tile_pool(name="pc_sb", bufs=1) as pc:
                    # sums scratch [H*N] -> [128, 64] -> reciprocal -> [64, 128] -> flat row
                    sums_pk = pc.tile([128, H * NCH], f32, name="sums_pk")
                    nc.sync.dma_start(
                        out=sums_pk[:],
                        in_=sums_d[:].rearrange("(r c) -> c r", r=H * NCH))
                    rcp_all = pc.tile([128, H * NCH], f32, name="rcp_all")
                    nc.vector.reciprocal(out=rcp_all[:], in_=sums_pk[:])
                    rcpT_sb = pc.tile([64, 128], f32, name="rcpT_sb")
                    with tc.tile_pool(name="pc_ps0", bufs=1, space="PSUM") as pcp0:
                        rcpT_ps = pcp0.tile([64, 128], f32, space="PSUM", name="rcpT_ps")
                        nc.tensor.transpose(out=rcpT_ps[:], in_=rcp_all[:], identity=ident[:])
                        nc.vector.tensor_copy(out=rcpT_sb[:], in_=rcpT_ps[:])
                    rcpflat = pc.tile([1, 64 * 128], f32, name="rcpflat")
                    nc.sync.dma_start(
                        out=rcpflat[:].rearrange("p (r c) -> p r c", r=64),
                        in_=rcpT_sb[:])

                    # rcp1 broadcast; scale outT; s2
                    outTs = [pc.tile([128, N], MMDT, name=f"outTs{c}") for c in range(DCH)]
                    s2cols = pc.tile([128, DCH], f32, name="s2cols")
                    scr2 = pc.tile([128, N], f32, name="scr2")
                    with tc.tile_pool(name="pc_rb", bufs=2, space="PSUM") as pcrb:
                        for ch in range(DCH):
                            rb_ps = pcrb.tile([128, N], f32, space="PSUM", name="rb_ps", tag="rb")
                            for hh in range(2):
                                hcur = ch * 2 + hh
                                for i in range(NCH):
                                    r = hcur * NCH + i
                                    nc.tensor.matmul(
                                        out=rb_ps[hh * 64:hh * 64 + 64, i * 128:(i + 1) * 128],
                                        lhsT=ones64[:],
                                        rhs=rcpflat[:, r * 128:(r + 1) * 128],
                                        start=True, stop=True)
                            rb_sb = pc.tile([128, N], f32, name="rb_sb", tag="rbsb", bufs=2)
                            nc.vector.tensor_copy(out=rb_sb[:], in_=rb_ps[:])
                            outTn = pc.tile([128, N], f32, name="outTn", tag="outTn", bufs=2)
                            nc.vector.tensor_tensor(out=outTn[:], in0=outT_sb[ch][:],
                                                    in1=rb_sb[:], op=Alu.mult)
                            nc.vector.tensor_copy(out=outTs[ch][:], in_=outTn[:])
                            nc.vector.tensor_tensor(out=scr2[:], in0=outTn[:], in1=wsb_sb[:], op=Alu.mult)
                            nc.vector.reduce_sum(out=s2cols[:, ch:ch + 1], in_=scr2[:], axis=AX)

                    # s2 row + per-head compare table
                    s2row_sb = pc.tile([1, D], f32, name="s2row_sb")
                    with tc.tile_pool(name="pc_ps1", bufs=2, space="PSUM") as pcp1:
                        for j in range(DCH):
                            s2row_ps = pcp1.tile([1, 128], f32, space="PSUM", name="s2row_ps", tag="s2r")
                            nc.tensor.transpose(out=s2row_ps[:], in_=s2cols[:, j:j + 1], identity=ident[:])
                            nc.vector.tensor_copy(out=s2row_sb[:, j * 128:(j + 1) * 128], in_=s2row_ps[:])

                    cnt2 = pc.tile([128, DCH], f32, name="cnt2")
                    mask2 = pc.tile([128, DCH], f32, name="mask2")
                    g2f = pc.tile([128, DCH], f32, name="g2f")
                    g2i = pc.tile([128, DCH], i32, name="g2i")
                    scr64 = pc.tile([128, 64], f32, name="scr64")
                    r2add = pc.tile([128, DCH], f32, name="r2add")
                    with tc.tile_pool(name="pc_t2", bufs=2, space="PSUM") as pct2:
                        for ch in range(DCH):
                            t2_ps = pct2.tile([128, 64], f32, space="PSUM", name="t2_ps", tag="t2")
                            for hh in range(2):
                                hcur = ch * 2 + hh
                                nc.tensor.matmul(out=t2_ps[hh * 64:hh * 64 + 64, :],
                                                 lhsT=ones64[:],
                                                 rhs=s2row_sb[:, hcur * 64:(hcur + 1) * 64],
                                                 start=True, stop=True)
                            nc.vector.tensor_tensor(
                                out=scr64[:], in0=t2_ps[:],
                                in1=s2cols[:, ch:ch + 1].to_broadcast([128, 64]), op=Alu.is_gt)
                            nc.vector.reduce_sum(out=cnt2[:, ch:ch + 1], in_=scr64[:], axis=AX)
                            nc.vector.tensor_scalar(out=mask2[:, ch:ch + 1], in0=cnt2[:, ch:ch + 1],
                                                    scalar1=float(K2), scalar2=None, op0=Alu.is_lt)

                    with tc.tile_pool(name="pc_ps2", bufs=1, space="PSUM") as pcp2:
                        r2_ps = pcp2.tile([128, DCH], f32, space="PSUM", name="r2_ps")
                        mask2i = pc.tile([128, DCH], i32, name="mask2i")
                        nc.vector.tensor_copy(out=mask2i[:], in_=mask2[:])
                        for ch in range(DCH):
                            nc.tensor.matmul(out=r2_ps[:, ch:ch + 1],
                                             lhsT=lt2_sb[:, ch, ch * 128:(ch + 1) * 128],
                                             rhs=mask2[:, ch:ch + 1],
                                             start=True, stop=True)
                            nc.vector.tensor_tensor(out=r2add[:, ch:ch + 1], in0=r2_ps[:, ch:ch + 1],
                                                    in1=hb_sb[:, ch:ch + 1], op=Alu.add)
                            nc.vector.select(out=g2f[:, ch:ch + 1], mask=mask2i[:, ch:ch + 1],
                                             on_true=r2add[:, ch:ch + 1], on_false=c408[:])
                        nc.vector.tensor_copy(out=g2i[:], in_=g2f[:])

                    wsel2_r = pc.tile([128, DCH, D], MMDT, name="wsel2_r")
                    with tc.tile_pool(name="pc_gt", bufs=2) as pcgt:
                        for j in range(DCH):
                            gtmp2 = pcgt.tile([128, D], f32, name="gtmp2", tag="gtmp2")
                            nc.gpsimd.indirect_dma_start(
                                out=gtmp2[:], out_offset=None,
                                in_=wout_d[:],
                                in_offset=IndirectOffsetOnAxis(ap=g2i[:, j:j + 1], axis=0))
                            nc.vector.tensor_copy(out=wsel2_r[:, j, :], in_=gtmp2[:])

                    # y = out2norm @ W_out_sel + b_out
                    with tc.tile_pool(name="pc_y", bufs=2, space="PSUM") as pcy:
                        for i in range(NCH):
                            y_ps = pcy.tile([128, D], f32, space="PSUM", name="y_ps", tag="y")
                            for j in range(DCH):
                                nc.tensor.matmul(out=y_ps[:],
                                                 lhsT=outTs[j][:, i * 128:(i + 1) * 128],
                                                 rhs=wsel2_r[:, j, :],
                                                 start=(j == 0), stop=(j == DCH - 1))
                            y_sb = pc.tile([128, D], f32, name="y_sb", tag="ysb", bufs=2)
                            nc.vector.tensor_tensor(out=y_sb[:], in0=y_ps[:], in1=boutb_sb[:], op=Alu.add)
                            nc.gpsimd.dma_start(out=y_d[i * 128:(i + 1) * 128, :], in_=y_sb[:])

                pool_expb.__exit__(None, None, None)

    nc.compile()
    return nc


def _host_prep(W_scoresx, W_qkv, rpb_table, headsita, W_scores, W_out, b_out, rel_index, dis):
    """Precompute batch-independent tensors on the host."""
    f = np.float32
    rpb = rpb_table[rel_index.reshape(-1)].reshape(N, N, H).transpose(2, 0, 1).astype(np.float64)
    factor = 1.0 / (2.0 * headsita.astype(np.float64) ** 2 + 1e-10)
    pos = np.exp(-factor[:, None, None] * dis.astype(np.float64)[None, :, :])
    expb = np.exp(rpb + 0.01 * pos).astype(f)

    wqkv_pad = np.zeros((KD + 1, 3 * D), f)
    wqkv_pad[:KD] = W_qkv
    wout_pad = np.zeros((H * K2 + 1, D), f)
    wout_pad[:H * K2] = W_out
    wsb = np.broadcast_to(W_scores.reshape(1, N), (128, N)).astype(f).copy()
    boutb = np.broadcast_to(b_out.reshape(1, D), (128, D)).astype(f).copy()
    lt1 = np.triu(np.ones((D, D), f), k=1)          # lt1[c', c] = 1 if c' < c
    lt2 = np.zeros((D, D), f)
    for h in range(H):
        lt2[h * DH:(h + 1) * DH, h * DH:(h + 1) * DH] = np.triu(np.ones((DH, DH), f), k=1)
    hb = np.zeros((128, DCH), f)
    for p in range(128):
        for j in range(DCH):
            hb[p, j] = ((j * 128 + p) // DH) * K2
    return {
        "wsx": W_scoresx.astype(f).reshape(N, 1),
        "wqkv_pad": wqkv_pad, "wout_pad": wout_pad, "expb": expb,
        "wscores_b": wsb, "bout_b": boutb, "lt1": lt1, "lt2": lt2, "hb": hb,
        "idt": np.eye(128, dtype=f),
    }


def kernel(x, W_scoresx, b_scoresx, W_qkv, rpb_table, headsita, W_scores, b_scores,
           W_out, b_out, rel_index, dis, _trace=False):
    from concourse.bass_utils import run_bass_kernel_spmd

    x = np.ascontiguousarray(np.asarray(x, dtype=np.float32))
    shared = _host_prep(np.asarray(W_scoresx, np.float32), np.asarray(W_qkv, np.float32),
                        np.asarray(rpb_table, np.float32), np.asarray(headsita, np.float32),
                        np.asarray(W_scores, np.float32), np.asarray(W_out, np.float32),
                        np.asarray(b_out, np.float32), np.asarray(rel_index),
                        np.asarray(dis, np.float32))

    if "nc" not in _CACHE:
        _CACHE["nc"] = _build_nc()
    nc = _CACHE["nc"]

    in_maps = []
    for c in range(B):
        m = dict(shared)
        m["x"] = np.ascontiguousarray(x[c])
        in_maps.append(m)
    res = run_bass_kernel_spmd(nc, in_maps, core_ids=list(range(B)), trace=_trace)
    y = np.stack([res.results[c]["y"] for c in range(B)])
    attn0 = np.stack([res.results[c]["attn0"] for c in range(B)])
    if _trace:
        _CACHE["last_result"] = res
    return y, attn0


# revision 34
# speedup vs baseline: 1.1473x; 1.0073x over previous
"""Trainium2 Bass kernel for nn_AttentionVTP (8-core batch-parallel).

Per core = one batch element. Two outputs: y [b,n,512] and softmax(dots0) [b,h,n,n].
Host precomputes expB = exp(rpb + 0.01*pos) (batch-independent); device does
scores->top-k masks->weight gathers, qkv, attention with fused softmax, second
pruning, output projection.
"""
import numpy as np

B, N, D, H, DH = 8, 1024, 512, 8, 64
KD = 409           # kept input channels after pruning 1
K2 = 51            # kept per-head channels after pruning 2
SCALE = DH ** -0.5
NCH = N // 128     # 8 n-chunks
DCH = D // 128     # 4 d-chunks
GI = 2             # i-blocks (of 128 query rows) per attention group
NGRP = NCH // GI

MM_MODE = "f32r"   # "f32" (exact, 4cyc/row) or "f32r" (tf32-ish, 1cyc/row @N>=256)

_CACHE = {}


def _build_nc():
    import concourse.mybir as mybir
    import concourse.tile as tile
    from concourse import bacc
    from concourse.bass import IndirectOffsetOnAxis

    f32 = mybir.dt.float32
    i32 = mybir.dt.int32
    MMDT = mybir.dt.float32r if MM_MODE == "f32r" else f32
    Alu = mybir.AluOpType
    Act = mybir.ActivationFunctionType
    AX = mybir.AxisListType.X

    nc = bacc.Bacc(None, target_bir_lowering=False, debug=False)

    with tile.TileContext(nc) as tc:
        with tc.tile_pool(name="dram", bufs=1, space="DRAM") as dram:
            def din(name, shape, dt=f32):
                return dram.tile(shape, dt, kind="ExternalInput", name=name, uniquify=False)

            x_d = din("x", [N, D])
            wsx_d = din("wsx", [N, 1])
            wqkv_d = din("wqkv_pad", [KD + 1, 3 * D])
            wout_d = din("wout_pad", [H * K2 + 1, D])
            expb_d = din("expb", [H, N, N])
            wsb_d = din("wscores_b", [128, N])
            boutb_d = din("bout_b", [128, D])
            lt1_d = din("lt1", [D, D])
            lt2_d = din("lt2", [D, D])
            hb_d = din("hb", [128, DCH])
            idt_d = din("idt", [128, 128])
            y_d = dram.tile([N, D], f32, kind="ExternalOutput", name="y", uniquify=False)
            a0_d = dram.tile([H, N, N], f32, kind="ExternalOutput", name="attn0", uniquify=False)
            sums_d = dram.tile([H * N], f32, kind="Internal", name="sums_scratch")

            with tc.tile_pool(name="persist", bufs=1) as pp:
                # ---- persistent SBUF ----
                wsx_sb = pp.tile([128, NCH], f32, name="wsx_sb")
                nc.sync.dma_start(out=wsx_sb[:], in_=wsx_d[:].rearrange("(i p) o -> p (i o)", p=128))
                wsb_sb = pp.tile([128, N], f32, name="wsb_sb")
                nc.sync.dma_start(out=wsb_sb[:], in_=wsb_d[:])
                boutb_sb = pp.tile([128, D], f32, name="boutb_sb")
                nc.sync.dma_start(out=boutb_sb[:], in_=boutb_d[:])
                hb_sb = pp.tile([128, DCH], f32, name="hb_sb")
                nc.sync.dma_start(out=hb_sb[:], in_=hb_d[:])
                ident = pp.tile([128, 128], f32, name="ident")
                nc.sync.dma_start(out=ident[:], in_=idt_d[:])
                lt2_sb = pp.tile([128, DCH, D], f32, name="lt2_sb")
                nc.sync.dma_start(out=lt2_sb[:], in_=lt2_d[:].rearrange("(j p) c -> p j c", p=128))

                ones64 = pp.tile([1, 64], f32, name="ones64")
                nc.vector.memset(ones64[:], 1.0)
                c409 = pp.tile([128, 1], f32, name="c409")
                nc.vector.memset(c409[:], float(KD))
                c408 = pp.tile([128, 1], f32, name="c408")
                nc.vector.memset(c408[:], float(H * K2))
                c1 = pp.tile([128, 1], f32, name="c1")
                nc.vector.memset(c1[:], 1.0)

                qkT_sb = [pp.tile([128, N], MMDT, name=f"qkT{m}") for m in range(8)]
                # v with a ones column per head: [nk-chunk][128, H, DH+1]
                v_sb = [pp.tile([128, H, DH + 1], MMDT, name=f"v{i}") for i in range(NCH)]
                outT_sb = [pp.tile([128, N], f32, name=f"outT{c}") for c in range(DCH)]
                sums0 = pp.tile([128, H * NCH], f32, name="sums0")

                # early pool for expb prefetch (lowest addresses -> no false dep on phase A)
                pool_expb = tc.tile_pool(name="pb_expb", bufs=3)
                pbx = pool_expb.__enter__()

                # ================= Phase A =================
                with tc.tile_pool(name="pa_sb", bufs=1) as pa:
                    x_sb = pa.tile([128, NCH, D], f32, name="x_sb")
                    nc.sync.dma_start(out=x_sb[:], in_=x_d[:].rearrange("(i p) d -> p i d", p=128))
                    lt1_sb = pa.tile([128, DCH, D], f32, name="lt1_sb")
                    nc.sync.dma_start(out=lt1_sb[:], in_=lt1_d[:].rearrange("(j p) c -> p j c", p=128))

                    with tc.tile_pool(name="pa_ps1", bufs=1, space="PSUM") as pap:
                        # s1[c] = sum_n x[n, c] * wsx[n]
                        s1_ps = pap.tile([128, DCH], f32, space="PSUM", name="s1_ps")
                        for j in range(DCH):
                            for i in range(NCH):
                                nc.tensor.matmul(out=s1_ps[:, j:j + 1],
                                                 lhsT=x_sb[:, i, j * 128:(j + 1) * 128],
                                                 rhs=wsx_sb[:, i:i + 1],
                                                 start=(i == 0), stop=(i == NCH - 1))
                        s1_sb = pa.tile([128, DCH], f32, name="s1_sb")
                        nc.vector.tensor_copy(out=s1_sb[:], in_=s1_ps[:])

                        # s1 as a broadcast row [128, 512]
                        s1row_sb = pa.tile([1, D], f32, name="s1row_sb")
                        for j in range(DCH):
                            row_ps = pap.tile([1, 128], f32, space="PSUM", name="row_ps", tag="row", bufs=2)
                            nc.tensor.transpose(out=row_ps[:], in_=s1_sb[:, j:j + 1], identity=ident[:])
                            nc.vector.tensor_copy(out=s1row_sb[:, j * 128:(j + 1) * 128], in_=row_ps[:])
                        ones128 = pa.tile([1, 128], f32, name="ones128")
                        nc.vector.memset(ones128[:], 1.0)
                        s1rb_ps = pap.tile([128, D], f32, space="PSUM", name="s1rb_ps")
                        nc.tensor.matmul(out=s1rb_ps[:], lhsT=ones128[:], rhs=s1row_sb[:], start=True, stop=True)
                        s1rb_sb = pa.tile([128, D], f32, name="s1rb_sb")
                        nc.vector.tensor_copy(out=s1rb_sb[:], in_=s1rb_ps[:])

                        # greater-count -> mask1
                        scratch = pa.tile([128, D], f32, name="scratch")
                        cnt_sb = pa.tile([128, DCH], f32, name="cnt_sb")
                        mask1 = pa.tile([128, DCH], f32, name="mask1")
                        for j in range(DCH):
                            nc.vector.tensor_tensor(
                                out=scratch[:], in0=s1rb_sb[:],
                                in1=s1_sb[:, j:j + 1].to_broadcast([128, D]), op=Alu.is_gt)
                            nc.vector.reduce_sum(out=cnt_sb[:, j:j + 1], in_=scratch[:], axis=AX)
                            nc.vector.tensor_scalar(out=mask1[:, j:j + 1], in0=cnt_sb[:, j:j + 1],
                                                    scalar1=float(KD), scalar2=None, op0=Alu.is_lt)

                        # rank1 = strict-lower-tri @ mask1 ; g1 = select(mask1, rank1, 409)
                        r1_ps = pap.tile([128, DCH], f32, space="PSUM", name="r1_ps")
                        mask1i = pa.tile([128, DCH], i32, name="mask1i")
                        nc.vector.tensor_copy(out=mask1i[:], in_=mask1[:])
                        g1f = pa.tile([128, DCH], f32, name="g1f")
                        g1i = pa.tile([128, DCH], i32, name="g1i")
                        for i in range(DCH):
                            for j in range(i + 1):
                                nc.tensor.matmul(out=r1_ps[:, i:i + 1],
                                                 lhsT=lt1_sb[:, j, i * 128:(i + 1) * 128],
                                                 rhs=mask1[:, j:j + 1],
                                                 start=(j == 0), stop=(j == i))
                            nc.vector.select(out=g1f[:, i:i + 1], mask=mask1i[:, i:i + 1],
                                             on_true=r1_ps[:, i:i + 1], on_false=c409[:])
                        nc.vector.tensor_copy(out=g1i[:], in_=g1f[:])

                    # gather W_sel rows (into f32 temp, round-copy to MMDT)
                    wsel_r = pa.tile([128, DCH, 3 * D], MMDT, name="wsel_r")
                    with tc.tile_pool(name="pa_gt", bufs=2) as pagt:
                        for j in range(DCH):
                            gtmp = pagt.tile([128, 3 * D], f32, name="gtmp", tag="gtmp")
                            nc.gpsimd.indirect_dma_start(
                                out=gtmp[:], out_offset=None,
                                in_=wqkv_d[:],
                                in_offset=IndirectOffsetOnAxis(ap=g1i[:, j:j + 1], axis=0))
                            nc.vector.tensor_copy(out=wsel_r[:, j, :], in_=gtmp[:])

                    # xT tiles [d, n]
                    xT_sb = pa.tile([128, DCH, N], MMDT, name="xT_sb")
                    with tc.tile_pool(name="pa_tp", bufs=2, space="PSUM") as patp:
                        for i in range(NCH):
                            for j in range(DCH):
                                tp = patp.tile([128, 128], f32, space="PSUM", name="tp")
                                nc.tensor.transpose(out=tp[:], in_=x_sb[:, i, j * 128:(j + 1) * 128],
                                                    identity=ident[:])
                                if (i * DCH + j) % 2 == 0:
                                    nc.vector.tensor_copy(out=xT_sb[:, j, i * 128:(i + 1) * 128], in_=tp[:])
                                else:
                                    nc.scalar.copy(out=xT_sb[:, j, i * 128:(i + 1) * 128], in_=tp[:])

                    # qT/kT: [e, n] = sum_d wsel[d, e] * xT[d, n]
                    with tc.tile_pool(name="pa_mm", bufs=2, space="PSUM") as pam:
                        for m in range(8):
                            for half in range(2):
                                mm_ps = pam.tile([128, 512], f32, space="PSUM", name="mm_ps")
                                for j in range(DCH):
                                    nc.tensor.matmul(out=mm_ps[:],
                                                     lhsT=wsel_r[:, j, m * 128:(m + 1) * 128],
                                                     rhs=xT_sb[:, j, half * 512:(half + 1) * 512],
                                                     start=(j == 0), stop=(j == DCH - 1))
                                if m % 2 == 0:
                                    nc.vector.tensor_copy(out=qkT_sb[m][:, half * 512:(half + 1) * 512], in_=mm_ps[:])
                                else:
                                    nc.scalar.copy(out=qkT_sb[m][:, half * 512:(half + 1) * 512], in_=mm_ps[:])
                        # v natural: [n, dh_all]; write into [128, H, DH+1] layout + ones col
                        for i in range(NCH):
                            mm_ps = pam.tile([128, 512], f32, space="PSUM", name="mm_ps")
                            for j in range(DCH):
                                nc.tensor.matmul(out=mm_ps[:],
                                                 lhsT=xT_sb[:, j, i * 128:(i + 1) * 128],
                                                 rhs=wsel_r[:, j, 2 * D:3 * D],
                                                 start=(j == 0), stop=(j == DCH - 1))
                            if i % 2 == 0:
                                nc.vector.tensor_copy(out=v_sb[i][:, :, 0:DH],
                                                      in_=mm_ps[:].rearrange("p (h e) -> p h e", h=H))
                            else:
                                nc.scalar.copy(out=v_sb[i][:, :, 0:DH],
                                               in_=mm_ps[:].rearrange("p (h e) -> p h e", h=H))
                            nc.vector.tensor_copy(out=v_sb[i][:, :, DH:DH + 1],
                                                  in_=c1[:].to_broadcast([128, H, 1]))

                # ================= Phase B: attention =================
                def qT(h):
                    t = qkT_sb[h // 2]
                    p0 = (h % 2) * 64
                    return t[p0:p0 + 64, :]

                def kT(h):
                    t = qkT_sb[4 + h // 2]
                    p0 = (h % 2) * 64
                    return t[p0:p0 + 64, :]

                with tc.tile_pool(name="pb_sb", bufs=2) as pb, \
                     tc.tile_pool(name="pb_e1t", bufs=2) as pbe, \
                     tc.tile_pool(name="pb_dots", bufs=2, space="PSUM") as pbd, \
                     tc.tile_pool(name="pb_tp", bufs=2, space="PSUM") as pbt, \
                     tc.tile_pool(name="pb_av", bufs=2, space="PSUM") as pba:
                    cpi = 0
                    for h in range(H):
                        for g in range(NGRP):
                            expb_t = pbx.tile([128, GI, N], f32, name="expb_t", tag="expb")
                            nc.sync.dma_start(
                                out=expb_t[:],
                                in_=expb_d[h, g * GI * 128:(g + 1) * GI * 128, :]
                                    .rearrange("(ii p) m -> p ii m", p=128))
                            e0_t = pb.tile([128, GI, N], f32, name="e0_t", tag="e0", bufs=3)
                            e1_t = pb.tile([128, GI, N], f32, name="e1_t", tag="e1", bufs=3)
                            a0_t = pb.tile([128, GI, N], f32, name="a0_t", tag="a0", bufs=3)
                            rcp0_t = pb.tile([128, GI], f32, name="rcp0_t", tag="rcp0")
                            e1T_t = pbe.tile([128, NCH, GI * 128], MMDT, name="e1T_t", tag="e1T")

                            for ii in range(GI):
                                i = g * GI + ii
                                col = h * NCH + i
                                dots_ps = pbd.tile([128, N], f32, space="PSUM", name="dots_ps", tag="dots")
                                for half in range(2):
                                    nc.tensor.matmul(out=dots_ps[:, half * 512:(half + 1) * 512],
                                                     lhsT=qT(h)[:, i * 128:(i + 1) * 128],
                                                     rhs=kT(h)[:, half * 512:(half + 1) * 512],
                                                     start=True, stop=True)
                                nc.scalar.activation(out=e0_t[:, ii, :], in_=dots_ps[:],
                                                     func=Act.Exp, scale=SCALE,
                                                     accum_out=sums0[:, col:col + 1])
                                nc.vector.tensor_tensor(out=e1_t[:, ii, :], in0=e0_t[:, ii, :],
                                                        in1=expb_t[:, ii, :], op=Alu.mult)
                                nc.vector.reciprocal(out=rcp0_t[:, ii:ii + 1], in_=sums0[:, col:col + 1])
                                if (h * NGRP + g) % 2 == 0:
                                    nc.vector.tensor_scalar(out=a0_t[:, ii, :], in0=e0_t[:, ii, :],
                                                            scalar1=rcp0_t[:, ii:ii + 1], scalar2=None,
                                                            op0=Alu.mult)
                                else:
                                    nc.scalar.activation(out=a0_t[:, ii, :], in_=e0_t[:, ii, :],
                                                         func=Act.Copy, bias=0.0,
                                                         scale=rcp0_t[:, ii:ii + 1])
                                for kcg in range(2):
                                    tp = pbt.tile([128, 512], f32, space="PSUM", name="tpb", tag="tpb")
                                    for q in range(4):
                                        kc = kcg * 4 + q
                                        nc.tensor.transpose(
                                            out=tp[:, q * 128:(q + 1) * 128],
                                            in_=e1_t[:, ii, kc * 128:(kc + 1) * 128],
                                            identity=ident[:])
                                    dst = e1T_t[:, kcg * 4:(kcg + 1) * 4, ii * 128:(ii + 1) * 128]
                                    src = tp[:].rearrange("p (q c) -> p q c", q=4)
                                    if cpi % 2 == 0:
                                        nc.vector.tensor_copy(out=dst, in_=src)
                                    else:
                                        nc.scalar.copy(out=dst, in_=src)
                                    cpi += 1
                            # attn0 out via SWDGE
                            nc.gpsimd.dma_start(
                                out=a0_d[h, g * GI * 128:(g + 1) * GI * 128, :]
                                    .rearrange("(ii p) m -> p ii m", p=128),
                                in_=a0_t[:])
                            # attn @ v -> out^T [dh+1, nq-group]; row DH = sums1
                            av_ps = pba.tile([DH + 1, GI * 128], f32, space="PSUM", name="av_ps", tag="av")
                            for kc in range(NCH):
                                nc.tensor.matmul(out=av_ps[:],
                                                 lhsT=v_sb[kc][:, h, :],
                                                 rhs=e1T_t[:, kc, :],
                                                 start=(kc == 0), stop=(kc == NCH - 1))
                            p0 = (h % 2) * 64
                            nc.scalar.copy(out=outT_sb[h // 2][p0:p0 + 64, g * GI * 128:(g + 1) * GI * 128],
                                           in_=av_ps[0:DH, :])
                            sstage = pb.tile([1, GI * 128], f32, name="sstage", tag="sstage")
                            nc.scalar.copy(out=sstage[:], in_=av_ps[DH:DH + 1, :])
                            nc.gpsimd.dma_start(
                                out=sums_d[h * N + g * GI * 128:h * N + (g + 1) * GI * 128],
                                in_=sstage[:])

                # ================= Phase C =================
                with tc.tile_pool(name="pc_sb", bufs=1) as pc:
                    # sums scratch [H*N] -> [128, 64] -> reciprocal -> [64, 128] -> flat row
                    sums_pk = pc.tile([128, H * NCH], f32, name="sums_pk")
                    nc.sync.dma_start(
                        out=sums_pk[:],
                        in_=sums_d[:].rearrange("(r c) -> c r", r=H * NCH))
                    rcp_all = pc.tile([128, H * NCH], f32, name="rcp_all")
                    nc.vector.reciprocal(out=rcp_all[:], in_=sums_pk[:])
                    rcpT_sb = pc.tile([64, 128], f32, name="rcpT_sb")
                    with tc.tile_pool(name="pc_ps0", bufs=1, space="PSUM") as pcp0:
                        rcpT_ps = pcp0.tile([64, 128], f32, space="PSUM", name="rcpT_ps")
                        nc.tensor.transpose(out=rcpT_ps[:], in_=rcp_all[:], identity=ident[:])
                        nc.vector.tensor_copy(out=rcpT_sb[:], in_=rcpT_ps[:])
                    rcpflat = pc.tile([1, 64 * 128], f32, name="rcpflat")
                    nc.sync.dma_start(
                        out=rcpflat[:].rearrange("p (r c) -> p r c", r=64),
                        in_=rcpT_sb[:])

                    # rcp1 broadcast; scale outT; s2
                    outTs = [pc.tile([128, N], MMDT, name=f"outTs{c}") for c in range(DCH)]
                    s2cols = pc.tile([128, DCH], f32, name="s2cols")
                    scr2 = pc.tile([128, N], f32, name="scr2")
                    with tc.tile_pool(name="pc_rb", bufs=2, space="PSUM") as pcrb:
                        for ch in range(DCH):
                            rb_ps = pcrb.tile([128, N], f32, space="PSUM", name="rb_ps", tag="rb")
                            for hh in range(2):
                                hcur = ch * 2 + hh
                                for i in range(NCH):
                                    r = hcur * NCH + i
                                    nc.tensor.matmul(
                                        out=rb_ps[hh * 64:hh * 64 + 64, i * 128:(i + 1) * 128],
                                        lhsT=ones64[:],
                                        rhs=rcpflat[:, r * 128:(r + 1) * 128],
                                        start=True, stop=True)
                            rb_sb = pc.tile([128, N], f32, name="rb_sb", tag="rbsb", bufs=2)
                            nc.vector.tensor_copy(out=rb_sb[:], in_=rb_ps[:])
                            outTn = pc.tile([128, N], f32, name="outTn", tag="outTn", bufs=2)
                            nc.vector.tensor_tensor(out=outTn[:], in0=outT_sb[ch][:],
                                                    in1=rb_sb[:], op=Alu.mult)
                            nc.vector.tensor_copy(out=outTs[ch][:], in_=outTn[:])
                            nc.vector.tensor_tensor(out=scr2[:], in0=outTn[:], in1=wsb_sb[:], op=Alu.mult)
                            nc.vector.reduce_sum(out=s2cols[:, ch:ch + 1], in_=scr2[:], axis=AX)

                    # s2 row + per-head compare table
                    s2row_sb = pc.tile([1, D], f32, name="s2row_sb")
                    with tc.tile_pool(name="pc_ps1", bufs=2, space="PSUM") as pcp1:
                        for j in range(DCH):
                            s2row_ps = pcp1.tile([1, 128], f32, space="PSUM", name="s2row_ps", tag="s2r")
                            nc.tensor.transpose(out=s2row_ps[:], in_=s2cols[:, j:j + 1], identity=ident[:])
                            nc.vector.tensor_copy(out=s2row_sb[:, j * 128:(j + 1) * 128], in_=s2row_ps[:])

                    cnt2 = pc.tile([128, DCH], f32, name="cnt2")
                    mask2 = pc.tile([128, DCH], f32, name="mask2")
                    g2f = pc.tile([128, DCH], f32, name="g2f")
                    g2i = pc.tile([128, DCH], i32, name="g2i")
                    scr64 = pc.tile([128, 64], f32, name="scr64")
                    r2add = pc.tile([128, DCH], f32, name="r2add")
                    with tc.tile_pool(name="pc_t2", bufs=2, space="PSUM") as pct2:
                        for ch in range(DCH):
                            t2_ps = pct2.tile([128, 64], f32, space="PSUM", name="t2_ps", tag="t2")
                            for hh in range(2):
                                hcur = ch * 2 + hh
                                nc.tensor.matmul(out=t2_ps[hh * 64:hh * 64 + 64, :],
                                                 lhsT=ones64[:],
                                                 rhs=s2row_sb[:, hcur * 64:(hcur + 1) * 64],
                                                 start=True, stop=True)
                            nc.vector.tensor_tensor(
                                out=scr64[:], in0=t2_ps[:],
                                in1=s2cols[:, ch:ch + 1].to_broadcast([128, 64]), op=Alu.is_gt)
                            nc.vector.reduce_sum(out=cnt2[:, ch:ch + 1], in_=scr64[:], axis=AX)
                            nc.vector.tensor_scalar(out=mask2[:, ch:ch + 1], in0=cnt2[:, ch:ch + 1],
                                                    scalar1=float(K2), scalar2=None, op0=Alu.is_lt)

                    with tc.tile_pool(name="pc_ps2", bufs=1, space="PSUM") as pcp2:
                        r2_ps = pcp2.tile([128, DCH], f32, space="PSUM", name="r2_ps")
                        mask2i = pc.tile([128, DCH], i32, name="mask2i")
                        nc.vector.tensor_copy(out=mask2i[:], in_=mask2[:])
                        for ch in range(DCH):
                            nc.tensor.matmul(out=r2_ps[:, ch:ch + 1],
                                             lhsT=lt2_sb[:, ch, ch * 128:(ch + 1) * 128],
                                             rhs=mask2[:, ch:ch + 1],
                                             start=True, stop=True)
                            nc.vector.tensor_tensor(out=r2add[:, ch:ch + 1], in0=r2_ps[:, ch:ch + 1],
                                                    in1=hb_sb[:, ch:ch + 1], op=Alu.add)
                            nc.vector.select(out=g2f[:, ch:ch + 1], mask=mask2i[:, ch:ch + 1],
                                             on_true=r2add[:, ch:ch + 1], on_false=c408[:])
                        nc.vector.tensor_copy(out=g2i[:], in_=g2f[:])

                    wsel2_r = pc.tile([128, DCH, D], MMDT, name="wsel2_r")
                    with tc.tile_pool(name="pc_gt", bufs=2) as pcgt:
                        for j in range(DCH):
                            gtmp2 = pcgt.tile([128, D], f32, name="gtmp2", tag="gtmp2")
                            nc.gpsimd.indirect_dma_start(
                                out=gtmp2[:], out_offset=None,
                                in_=wout_d[:],
                                in_offset=IndirectOffsetOnAxis(ap=g2i[:, j:j + 1], axis=0))
                            nc.vector.tensor_copy(out=wsel2_r[:, j, :], in_=gtmp2[:])

                    # y = out2norm @ W_out_sel + b_out
                    with tc.tile_pool(name="pc_y", bufs=2, space="PSUM") as pcy:
                        for i in range(NCH):
                            y_ps = pcy.tile([128, D], f32, space="PSUM", name="y_ps", tag="y")
                            for j in range(DCH):
                                nc.tensor.matmul(out=y_ps[:],
                                                 lhsT=outTs[j][:, i * 128:(i + 1) * 128],
                                                 rhs=wsel2_r[:, j, :],
                                                 start=(j == 0), stop=(j == DCH - 1))
                            y_sb = pc.tile([128, D], f32, name="y_sb", tag="ysb", bufs=2)
                            nc.vector.tensor_tensor(out=y_sb[:], in0=y_ps[:], in1=boutb_sb[:], op=Alu.add)
                            nc.gpsimd.dma_start(out=y_d[i * 128:(i + 1) * 128, :], in_=y_sb[:])

                pool_expb.__exit__(None, None, None)

    nc.compile()
    return nc


def _host_prep(W_scoresx, W_qkv, rpb_table, headsita, W_scores, W_out, b_out, rel_index, dis):
    """Precompute batch-independent tensors on the host."""
    f = np.float32
    rpb = rpb_table[rel_index.reshape(-1)].reshape(N, N, H).transpose(2, 0, 1).astype(np.float64)
    factor = 1.0 / (2.0 * headsita.astype(np.float64) ** 2 + 1e-10)
    pos = np.exp(-factor[:, None, None] * dis.astype(np.float64)[None, :, :])
    expb = np.exp(rpb + 0.01 * pos).astype(f)

    wqkv_pad = np.zeros((KD + 1, 3 * D), f)
    wqkv_pad[:KD] = W_qkv
    wout_pad = np.zeros((H * K2 + 1, D), f)
    wout_pad[:H * K2] = W_out
    wsb = np.broadcast_to(W_scores.reshape(1, N), (128, N)).astype(f).copy()
    boutb = np.broadcast_to(b_out.reshape(1, D), (128, D)).astype(f).copy()
    lt1 = np.triu(np.ones((D, D), f), k=1)          # lt1[c', c] = 1 if c' < c
    lt2 = np.zeros((D, D), f)
    for h in range(H):
        lt2[h * DH:(h + 1) * DH, h * DH:(h + 1) * DH] = np.triu(np.ones((DH, DH), f), k=1)
    hb = np.zeros((128, DCH), f)
    for p in range(128):
        for j in range(DCH):
            hb[p, j] = ((j * 128 + p) // DH) * K2
    return {
        "wsx": W_scoresx.astype(f).reshape(N, 1),
        "wqkv_pad": wqkv_pad, "wout_pad": wout_pad, "expb": expb,
        "wscores_b": wsb, "bout_b": boutb, "lt1": lt1, "lt2": lt2, "hb": hb,
        "idt": np.eye(128, dtype=f),
    }


def kernel(x, W_scoresx, b_scoresx, W_qkv, rpb_table, headsita, W_scores, b_scores,
           W_out, b_out, rel_index, dis, _trace=False):
    from concourse.bass_utils import run_bass_kernel_spmd

    x = np.ascontiguousarray(np.asarray(x, dtype=np.float32))
    shared = _host_prep(np.asarray(W_scoresx, np.float32), np.asarray(W_qkv, np.float32),
                        np.asarray(rpb_table, np.float32), np.asarray(headsita, np.float32),
                        np.asarray(W_scores, np.float32), np.asarray(W_out, np.float32),
                        np.asarray(b_out, np.float32), np.asarray(rel_index),
                        np.asarray(dis, np.float32))

    if "nc" not in _CACHE:
        _CACHE["nc"] = _build_nc()
    nc = _CACHE["nc"]

    in_maps = []
    for c in range(B):
        m = dict(shared)
        m["x"] = np.ascontiguousarray(x[c])
        in_maps.append(m)
    res = run_bass_kernel_spmd(nc, in_maps, core_ids=list(range(B)), trace=_trace)
    y = np.stack([res.results[c]["y"] for c in range(B)])
    attn0 = np.stack([res.results[c]["attn0"] for c in range(B)])
    if _trace:
        _CACHE["last_result"] = res
    return y, attn0
